# revision 13
# baseline (speedup 1.0000x reference)
"""Trainium2 Bass kernel for nn_Dyanmic_Q_MLP (fake-quant MLP).

Computation (reference):
    w1q = fake_quant(w1, 8); w2q = fake_quant(w2, 8)       # per-tensor symmetric
    h   = relu(x @ w1q.T + b1)                             # [B,S,3072]
    out = h @ w2q.T + b2                                   # [B,S,768]

Strategy:
  * Data-parallel over the flattened (B*S)=12544 rows across 8 NeuronCores
    (1568 rows/core, zero-padded to 1664 = 13*128). Weights replicated.
    No collectives.
  * Host side only reshapes/transposes/shards (layout, no math):
      xt  = x.T slice per core   [768, 1664]
      w1t = w1.T                 [768, 3072]
      w2t = w2.T                 [3072, 768]
  * On-device fake-quant: global abs-max (DVE reduce + GPSIMD partition
    all-reduce), scale = max/qmax, integer-valued weights q = round(w/scale)
    via the +-1.5*2^23 RNE trick.  q in [-127,127] is EXACTLY representable
    in bf16, so matmuls run on the bf16 PE path with no weight error; the
    scale is folded into the epilogues (relu(s1*z+b1) = s1*relu(z+b1/s1)).
  * Activations are split hi/lo into two bf16 operands (x = hi + lo), giving
    ~fp32-accurate matmuls at 2x bf16 cost (vs 4x for native fp32 path).
"""

import sys

for _p in ("/opt/trn_rl_repo", "/root/.axon_site/_ro/trn_rl_repo"):
    if _p not in sys.path:
        sys.path.insert(0, _p)

from contextlib import ExitStack

import numpy as np

import concourse.bass as bass
import concourse.mybir as mybir
import concourse.tile as tile
from concourse import bass_utils

N_CORES = 8
B, S, D, H = 64, 196, 768, 3072
M_TOTAL = B * S            # 12544
M_SHARD = M_TOTAL // N_CORES   # 1568
M_PAD = 1664               # 13 * 128
M_BLOCKS = [384, 384, 384, 384, 128]
KD = D // 128              # 6
KH = H // 128              # 24
C_RNE = 12582912.0         # 1.5 * 2**23: (v + C) - C == round-to-nearest-even(v)
USE_SPLIT = True           # hi/lo bf16 split of activations (pseudo-fp32)

F32 = mybir.dt.float32
BF16 = mybir.dt.bfloat16
ALU = mybir.AluOpType
ACTF = mybir.ActivationFunctionType


def _split_oversized_waits(nc, max_waits=1):
    """The walrus build in this container accepts only one sync-wait per
    instruction.  Hoist excess on_wait entries onto inserted same-engine
    NoOp instructions placed just before (queue-order preserves semantics;
    a NoOp-with-wait stalls the queue without flushing the engine pipe)."""
    for f in nc.m.functions:
        for b in f.blocks:
            new_list, changed, ctr = [], False, 0
            for i in b.instructions:
                si = i.sync_info
                w = list(si.on_wait) if si is not None else []
                if len(w) > max_waits:
                    extra, keep = w[:-max_waits], w[-max_waits:]
                    for ci in range(0, len(extra), max_waits):
                        ctr += 1
                        d = mybir.InstNoOp(
                            name=f"{i.name}-wsplit{ctr}",
                            engine=i.engine,
                        )
                        d.sync_info = mybir.SyncInfo(
                            on_update=[], on_wait=extra[ci : ci + max_waits]
                        )
                        new_list.append(d)
                    si.on_wait = keep
                    changed = True
                new_list.append(i)
            if changed:
                b.instructions = new_list


def build_program(qmax: float, use_split: bool = USE_SPLIT, walrus_fixups: bool = True):
    """Build the per-core Bass program (same NEFF on all 8 cores).

    walrus_fixups: apply _split_oversized_waits (needed for the walrus
    compile; CoreSim chokes on the inserted drains, so sim callers pass
    False)."""
    nc = bass.Bass("TRN2", target_bir_lowering=False, debug=False)

    xt_d = nc.dram_tensor("xt", (D, M_PAD), F32, kind="ExternalInput").ap()
    w1t_d = nc.dram_tensor("w1t", (D, H), F32, kind="ExternalInput").ap()
    w2t_d = nc.dram_tensor("w2t", (H, D), F32, kind="ExternalInput").ap()
    # b1 comes host-side pre-packed as [128, KH]: column t holds
    # b1[t*128:(t+1)*128]; b2 comes pre-broadcast as [128, D].
    b1_d = nc.dram_tensor("b1", (128, KH), F32, kind="ExternalInput").ap()
    b2_d = nc.dram_tensor("b2", (128, D), F32, kind="ExternalInput").ap()
    out_d = nc.dram_tensor("out", (M_PAD, D), F32, kind="ExternalOutput").ap()

    with tile.TileContext(nc) as tc, ExitStack() as ctx:
        const = ctx.enter_context(tc.tile_pool(name="const", bufs=1))
        wq = ctx.enter_context(tc.tile_pool(name="wq", bufs=1))
        wstage = ctx.enter_context(tc.tile_pool(name="wstage", bufs=2))
        xstage = ctx.enter_context(tc.tile_pool(name="xstage", bufs=2))
        xsplit = ctx.enter_context(tc.tile_pool(name="xsplit", bufs=2))
        hpool = ctx.enter_context(tc.tile_pool(name="hpool", bufs=1))
        hf32p = ctx.enter_context(tc.tile_pool(name="hf32p", bufs=2))
        opool = ctx.enter_context(tc.tile_pool(name="opool", bufs=2))
        scal = ctx.enter_context(tc.tile_pool(name="scal", bufs=1))
        ps1 = ctx.enter_context(tc.tile_pool(name="ps1", bufs=4, space="PSUM"))
        ps2 = ctx.enter_context(tc.tile_pool(name="ps2", bufs=2, space="PSUM"))
        dram = ctx.enter_context(tc.tile_pool(name="dram", bufs=1, space="DRAM"))

        # ---------- setup: biases (already laid out by the host) ----------
        b1_pack = const.tile([128, KH], F32, tag="b1pack")
        nc.sync.dma_start(b1_pack[:], b1_d[:])
        b2_bc = const.tile([128, D], F32, tag="b2bc")
        nc.sync.dma_start(b2_bc[:], b2_d[:])

        def bcast_scalar(src, tag):
            """Replicate a [1,1] SBUF scalar to [128,1] via a DRAM hop
            (SBUF APs cannot have 0-step partition dims, DRAM APs can)."""
            scr = dram.tile([1, 1], F32, tag=f"{tag}scr", name=f"{tag}scr")
            nc.sync.dma_start(scr[:], src[:])
            bc = scal.tile([128, 1], F32, tag=f"{tag}bc", name=f"{tag}bc")
            nc.sync.dma_start(bc[:], scr.broadcast_to([128, 1]))
            return bc

        # ---------- fake-quant of weights ----------
        def quantize(wt_d, n_rows, free_dim, chunk, dst_tiles, tag):
            """Two passes over wt_d ([n_rows*128, free_dim] DRAM, row-major):
            pass1 computes the global abs-max, pass2 re-loads and writes
            round(w/scale) as bf16 integers into dst_tiles[k][:, :].
            Returns (scale[128,1], inv_scale[128,1]) tiles (same value in
            every partition)."""
            n_chunks_per_row = free_dim // chunk
            macc = scal.tile([128, 1], F32, tag=f"{tag}macc")
            first = True
            for k in range(n_rows):
                for j in range(n_chunks_per_row):
                    wst = wstage.tile([128, chunk], F32, tag=f"{tag}st")
                    nc.sync.dma_start(
                        wst[:], wt_d[k * 128 : (k + 1) * 128, j * chunk : (j + 1) * chunk]
                    )
                    mk = scal.tile([128, 1], F32, tag=f"{tag}mk", bufs=2)
                    nc.vector.tensor_reduce(
                        mk[:], wst[:], axis=mybir.AxisListType.X,
                        op=ALU.max, apply_absolute_value=True,
                    )
                    if first:
                        nc.vector.tensor_copy(macc[:], mk[:])
                        first = False
                    else:
                        nc.vector.tensor_tensor(macc[:], macc[:], mk[:], op=ALU.max)
            # cross-partition max: DMA-gather [128,1] -> [1,128], reduce
            row = scal.tile([1, 128], F32, tag=f"{tag}row", name=f"{tag}row")
            nc.sync.dma_start(row[:], macc[:])
            gmax = scal.tile([1, 1], F32, tag=f"{tag}gmax", name=f"{tag}gmax")
            nc.vector.tensor_reduce(gmax[:], row[:], axis=mybir.AxisListType.X, op=ALU.max)
            # walrus rejects ALU divide in tensor_scalar; mult by 1/qmax
            # differs from max/qmax by <=1 ulp (negligible: it only shifts
            # the global output scale by ~1e-7 relative).
            scale = scal.tile([1, 1], F32, tag=f"{tag}scale", name=f"{tag}scale")
            nc.vector.tensor_scalar(scale[:], gmax[:], 1.0 / float(qmax), None, op0=ALU.mult)
            inv1 = scal.tile([1, 1], F32, tag=f"{tag}inv1", name=f"{tag}inv1")
            nc.vector.reciprocal(inv1[:], scale[:])
            inv_s = bcast_scalar(inv1, f"{tag}inv")
            # pass 2: round(w * inv_s) -> bf16 (exact integers)
            for k in range(n_rows):
                for j in range(n_chunks_per_row):
                    wst2 = wstage.tile([128, chunk], F32, tag=f"{tag}st2")
                    nc.sync.dma_start(
                        wst2[:], wt_d[k * 128 : (k + 1) * 128, j * chunk : (j + 1) * chunk]
                    )
                    nc.vector.tensor_scalar(
                        wst2[:], wst2[:], inv_s[:], C_RNE, op0=ALU.mult, op1=ALU.add
                    )
                    nc.vector.tensor_scalar(
                        dst_tiles[k][:, j * chunk : (j + 1) * chunk],
                        wst2[:], C_RNE, None, op0=ALU.subtract,
                    )
            return scale, inv_s

        w1q = [wq.tile([128, H], BF16, tag=f"w1q{d}", name=f"w1q{d}") for d in range(KD)]
        w2q = [wq.tile([128, D], BF16, tag=f"w2q{t}", name=f"w2q{t}") for t in range(KH)]

        s1, inv_s1 = quantize(w1t_d, KD, H, 1024, w1q, "q1")
        s2, _ = quantize(w2t_d, KH, D, D, w2q, "q2")

        # b1' = b1 / s1   (per-partition column layout [128, KH])
        b1s = const.tile([128, KH], F32, tag="b1s")
        nc.vector.tensor_scalar(b1s[:], b1_pack[:], inv_s1[:], None, op0=ALU.mult)
        # c = s1 * s2  (final output scale), replicated to [128,1]
        c11 = scal.tile([1, 1], F32, tag="c11")
        nc.vector.tensor_tensor(c11[:], s1[:], s2[:], op=ALU.mult)
        cscale = bcast_scalar(c11, "c")

        # ---------- main pipeline over row blocks ----------
        m0 = 0
        for m_blk in M_BLOCKS:
            # load x block (transposed layout) and split into bf16 hi/lo
            xh, xl = [], []
            for d in range(KD):
                xs_ = xstage.tile([128, m_blk], F32, tag=f"xs{d}", name=f"xs{d}")
                nc.sync.dma_start(xs_[:], xt_d[d * 128 : (d + 1) * 128, m0 : m0 + m_blk])
                xh_ = xsplit.tile([128, m_blk], BF16, tag=f"xh{d}", name=f"xh{d}")
                nc.vector.tensor_copy(xh_[:], xs_[:])
                xh.append(xh_)
                if use_split:
                    xl_ = xsplit.tile([128, m_blk], BF16, tag=f"xl{d}", name=f"xl{d}")
                    nc.vector.tensor_tensor(xl_[:], xs_[:], xh_[:], op=ALU.subtract)
                    xl.append(xl_)

            # fc1: hT[t] = relu(w1q.T-contract + b1')   (hidden on partitions)
            hh, hl = [], []
            for t in range(KH):
                ps = ps1.tile([128, m_blk], F32, tag="ps1", name="ps1")
                total = KD * (2 if use_split else 1)
                cnt = 0
                for d in range(KD):
                    lhs = w1q[d][:, t * 128 : (t + 1) * 128]
                    nc.tensor.matmul(
                        ps[:], lhs, xh[d][:], start=(cnt == 0), stop=(cnt == total - 1)
                    )
                    cnt += 1
                    if use_split:
                        nc.tensor.matmul(
                            ps[:], lhs, xl[d][:], start=False, stop=(cnt == total - 1)
                        )
                        cnt += 1
                hf = hf32p.tile([128, m_blk], F32, tag="hf", name="hf")
                nc.scalar.activation(hf[:], ps[:], ACTF.Relu, bias=b1s[:, t : t + 1])
                hh_ = hpool.tile([128, m_blk], BF16, tag=f"hh{t}", name=f"hh{t}")
                nc.scalar.activation(hh_[:], hf[:], ACTF.Copy)
                hh.append(hh_)
                if use_split:
                    hl_ = hpool.tile([128, m_blk], BF16, tag=f"hl{t}", name=f"hl{t}")
                    nc.vector.tensor_tensor(hl_[:], hf[:], hh_[:], op=ALU.subtract)
                    hl.append(hl_)

            # fc2: out[m, :] = c * (hT.T @ w2q) + b2
            parts = [hh, hl] if use_split else [hh]
            for ms in range(m_blk // 128):
                halves = []
                for h0, hw in ((0, 384), (384, 384)):
                    ps_ = ps2.tile([128, 384], F32, tag=f"ps2_{h0}", name=f"ps2_{h0}")
                    total = KH * len(parts)
                    cnt = 0
                    for t in range(KH):
                        for part in parts:
                            nc.tensor.matmul(
                                ps_[:],
                                part[t][:, ms * 128 : (ms + 1) * 128],
                                w2q[t][:, h0 : h0 + hw],
                                start=(cnt == 0),
                                stop=(cnt == total - 1),
                            )
                            cnt += 1
                    halves.append((h0, hw, ps_))
                ot = opool.tile([128, D], F32, tag="ot", name="ot")
                for h0, hw, ps_ in halves:
                    nc.vector.tensor_scalar(
                        ot[:, h0 : h0 + hw], ps_[:], cscale[:], None, op0=ALU.mult
                    )
                nc.vector.tensor_tensor(ot[:], ot[:], b2_bc[:], op=ALU.add)
                row = m0 + ms * 128
                nc.sync.dma_start(out_d[row : row + 128, :], ot[:])
            m0 += m_blk

    if walrus_fixups:
        _split_oversized_waits(nc)
    return nc


_PROGRAM_CACHE = {}


def _get_program(qmax: float, use_split: bool = USE_SPLIT):
    key = (qmax, use_split)
    if key not in _PROGRAM_CACHE:
        _PROGRAM_CACHE[key] = build_program(qmax, use_split)
    return _PROGRAM_CACHE[key]


def kernel(x, w1, b1, w2, b2, bits):
    qmax = float(2.0 ** (int(bits) - 1) - 1.0)
    nc = _get_program(qmax)

    x = np.ascontiguousarray(np.asarray(x, dtype=np.float32)).reshape(M_TOTAL, D)
    w1t = np.ascontiguousarray(np.asarray(w1, dtype=np.float32).T)   # [768, 3072]
    w2t = np.ascontiguousarray(np.asarray(w2, dtype=np.float32).T)   # [3072, 768]
    b1h = np.ascontiguousarray(
        np.asarray(b1, dtype=np.float32).reshape(KH, 128).T
    )  # [128, KH]
    b2h = np.ascontiguousarray(
        np.broadcast_to(np.asarray(b2, dtype=np.float32).reshape(1, D), (128, D))
    )  # [128, D]
    xt_full = np.ascontiguousarray(x.T)                              # [768, 12544]

    in_maps = []
    for c in range(N_CORES):
        xt_c = np.zeros((D, M_PAD), dtype=np.float32)
        xt_c[:, :M_SHARD] = xt_full[:, c * M_SHARD : (c + 1) * M_SHARD]
        in_maps.append(
            {"xt": xt_c, "w1t": w1t, "w2t": w2t, "b1": b1h, "b2": b2h}
        )

    res = bass_utils.run_bass_kernel_spmd(nc, in_maps, core_ids=list(range(N_CORES)))
    out = np.concatenate(
        [res.results[c]["out"][:M_SHARD] for c in range(N_CORES)], axis=0
    )
    return np.ascontiguousarray(out.reshape(B, S, D))


# revision 39
# speedup vs baseline: 1.1088x; 1.1088x over previous
"""Trainium2 Bass kernel for nn_Dyanmic_Q_MLP (fake-quant MLP).

Computation (reference):
    w1q = fake_quant(w1, 8); w2q = fake_quant(w2, 8)       # per-tensor symmetric
    h   = relu(x @ w1q.T + b1)                             # [B,S,3072]
    out = h @ w2q.T + b2                                   # [B,S,768]

Strategy:
  * Data-parallel over the flattened (B*S)=12544 rows across 8 NeuronCores
    (1568 rows/core, zero-padded to 1664 = 13*128). Weights replicated.
    No collectives.
  * Host side only reshapes/transposes/shards (layout, no math):
      xt  = x.T slice per core   [768, 1664]
      w1t = w1.T                 [768, 3072]
      w2t = w2.T                 [3072, 768]
  * On-device fake-quant: global abs-max (DVE reduce + GPSIMD partition
    all-reduce), scale = max/qmax, integer-valued weights q = round(w/scale)
    via the +-1.5*2^23 RNE trick.  q in [-127,127] is EXACTLY representable
    in bf16, so matmuls run on the bf16 PE path with no weight error; the
    scale is folded into the epilogues (relu(s1*z+b1) = s1*relu(z+b1/s1)).
  * Activations are split hi/lo into two bf16 operands (x = hi + lo), giving
    ~fp32-accurate matmuls at 2x bf16 cost (vs 4x for native fp32 path).
"""

import sys

for _p in ("/opt/trn_rl_repo", "/root/.axon_site/_ro/trn_rl_repo"):
    if _p not in sys.path:
        sys.path.insert(0, _p)

from contextlib import ExitStack

import numpy as np

import concourse.bass as bass
import concourse.mybir as mybir
import concourse.tile as tile
from concourse import bass_utils
from concourse.tile_rust import add_dep_helper

N_CORES = 8
B, S, D, H = 64, 196, 768, 3072
M_TOTAL = B * S            # 12544
M_SHARD = M_TOTAL // N_CORES   # 1568
M_PAD = M_SHARD            # no padding
# 6 blocks of 256 + a 32-row tail: 13 fc2 row-subtiles total (N-bound cost),
# small blocks let fc1 of block k+1 overlap fc2 of block k (h double-buffer)
M_BLOCKS = [256] * 6 + [32]
KD = D // 128              # 6
KH = H // 128              # 24
C_RNE = 12582912.0         # 1.5 * 2**23: (v + C) - C == round-to-nearest-even(v)
USE_SPLIT = True           # hi/lo bf16 split of activations (pseudo-fp32)

F32 = mybir.dt.float32
BF16 = mybir.dt.bfloat16
ALU = mybir.AluOpType
ACTF = mybir.ActivationFunctionType


def _split_oversized_waits(nc, max_waits=1):
    """The walrus build in this container accepts only one sync-wait per
    instruction.  Hoist excess on_wait entries onto inserted same-engine
    NoOp instructions placed just before (queue-order preserves semantics;
    a NoOp-with-wait stalls the queue without flushing the engine pipe)."""
    for f in nc.m.functions:
        for b in f.blocks:
            new_list, changed, ctr = [], False, 0
            for i in b.instructions:
                si = i.sync_info
                w = list(si.on_wait) if si is not None else []
                if len(w) > max_waits:
                    extra, keep = w[:-max_waits], w[-max_waits:]
                    for ci in range(0, len(extra), max_waits):
                        ctr += 1
                        d = mybir.InstNoOp(
                            name=f"{i.name}-wsplit{ctr}",
                            engine=i.engine,
                        )
                        d.sync_info = mybir.SyncInfo(
                            on_update=[], on_wait=extra[ci : ci + max_waits]
                        )
                        new_list.append(d)
                    si.on_wait = keep
                    changed = True
                new_list.append(i)
            if changed:
                b.instructions = new_list


def build_program(qmax: float, use_split: bool = USE_SPLIT, walrus_fixups: bool = True):
    """Build the per-core Bass program (same NEFF on all 8 cores).

    walrus_fixups: apply _split_oversized_waits (needed for the walrus
    compile; CoreSim chokes on the inserted drains, so sim callers pass
    False)."""
    nc = bass.Bass("TRN2", target_bir_lowering=False, debug=False)

    xt_d = nc.dram_tensor("xt", (D, M_PAD), F32, kind="ExternalInput").ap()
    w1t_d = nc.dram_tensor("w1t", (D, H), F32, kind="ExternalInput").ap()
    w2t_d = nc.dram_tensor("w2t", (H, D), F32, kind="ExternalInput").ap()
    # b1 comes host-side pre-packed as [128, KH]: column t holds
    # b1[t*128:(t+1)*128]; b2 comes pre-broadcast as [128, D].
    b1_d = nc.dram_tensor("b1", (128, KH), F32, kind="ExternalInput").ap()
    b2_d = nc.dram_tensor("b2", (128, D), F32, kind="ExternalInput").ap()
    out_d = nc.dram_tensor("out", (M_PAD, D), F32, kind="ExternalOutput").ap()

    with tile.TileContext(nc) as tc, ExitStack() as ctx:
        const = ctx.enter_context(tc.tile_pool(name="const", bufs=1))
        wq = ctx.enter_context(tc.tile_pool(name="wq", bufs=1))
        wstage = ctx.enter_context(tc.tile_pool(name="wstage", bufs=2))
        xstage = ctx.enter_context(tc.tile_pool(name="xstage", bufs=2))
        xsplit = ctx.enter_context(tc.tile_pool(name="xsplit", bufs=2))
        hpool = ctx.enter_context(tc.tile_pool(name="hpool", bufs=2))
        hf32p = ctx.enter_context(tc.tile_pool(name="hf32p", bufs=2))
        opool = ctx.enter_context(tc.tile_pool(name="opool", bufs=2))
        scal = ctx.enter_context(tc.tile_pool(name="scal", bufs=1))
        ps1 = ctx.enter_context(tc.tile_pool(name="ps1", bufs=4, space="PSUM"))
        ps2 = ctx.enter_context(tc.tile_pool(name="ps2", bufs=2, space="PSUM"))
        dram = ctx.enter_context(tc.tile_pool(name="dram", bufs=1, space="DRAM"))

        # ---------- setup: biases (already laid out by the host) ----------
        b1_pack = const.tile([128, KH], F32, tag="b1pack")
        nc.sync.dma_start(b1_pack[:], b1_d[:])
        b2_bc = const.tile([128, D], F32, tag="b2bc")
        nc.sync.dma_start(b2_bc[:], b2_d[:])

        # ---------- fake-quant of weights ----------
        def quantize(wt_d, n_rows, free_dim, chunk, dst_tiles, tag,
                     pass2_j_major=False, gate_pass1_on=None):
            """Two passes over wt_d ([n_rows*128, free_dim] DRAM, row-major):
            pass1 computes the global abs-max, pass2 re-loads and writes
            round(w/scale) as bf16 integers into dst_tiles[k][:, :].
            pass2_j_major orders pass-2 chunks column-block-major so the
            first matmuls (which need the leading columns of EVERY row
            tile) unblock as early as possible.
            Returns (scale[1,1], inv_scale[128,1]) tiles."""
            n_chunks_per_row = free_dim // chunk
            macc = scal.tile([128, 1], F32, tag=f"{tag}macc")
            first = True
            macc_last = None
            for k in range(n_rows):
                for j in range(n_chunks_per_row):
                    wst = wstage.tile([128, chunk], F32, tag=f"{tag}st", bufs=4)
                    dma = nc.sync.dma_start(
                        wst[:], wt_d[k * 128 : (k + 1) * 128, j * chunk : (j + 1) * chunk]
                    )
                    if gate_pass1_on is not None:
                        add_dep_helper(dma.ins, gate_pass1_on,
                                       reason="serialize bulk weight DMA streams")
                    mk = scal.tile([128, 1], F32, tag=f"{tag}mk", bufs=2)
                    nc.vector.tensor_reduce(
                        mk[:], wst[:], axis=mybir.AxisListType.X,
                        op=ALU.max, apply_absolute_value=True,
                    )
                    if first:
                        macc_last = nc.vector.tensor_copy(macc[:], mk[:])
                        first = False
                    else:
                        macc_last = nc.vector.tensor_tensor(
                            macc[:], macc[:], mk[:], op=ALU.max
                        )
            # cross-partition max, replicated to every partition in one
            # round-trip: macc[128,1] -> DRAM[128] -> (0-step partition
            # broadcast read) SBUF[128,128] -> free-dim reduce -> [128,1].
            # SWDGE keeps these latency-critical hops off the bulk HWDGE
            # queues.
            mscr = dram.tile([1, 128], F32, tag=f"{tag}mscr", name=f"{tag}mscr")
            nc.gpsimd.dma_start(mscr[:], macc[:])
            mall = scal.tile([128, 128], F32, tag=f"{tag}mall", name=f"{tag}mall")
            nc.gpsimd.dma_start(mall[:], mscr.broadcast_to([128, 128]))
            gmax = scal.tile([128, 1], F32, tag=f"{tag}gmax", name=f"{tag}gmax")
            nc.vector.tensor_reduce(gmax[:], mall[:], axis=mybir.AxisListType.X, op=ALU.max)
            # walrus rejects ALU divide in tensor_scalar; mult by 1/qmax
            # differs from max/qmax by <=1 ulp (negligible: it only shifts
            # the global output scale by ~1e-7 relative).
            scale = scal.tile([128, 1], F32, tag=f"{tag}scale", name=f"{tag}scale")
            nc.vector.tensor_scalar(scale[:], gmax[:], 1.0 / float(qmax), None, op0=ALU.mult)
            inv_s = scal.tile([128, 1], F32, tag=f"{tag}inv", name=f"{tag}inv")
            nc.vector.reciprocal(inv_s[:], scale[:])
            # pass 2: round(w * inv_s) -> bf16 (exact integers)
            if pass2_j_major:
                order = [(k, j) for j in range(n_chunks_per_row) for k in range(n_rows)]
            else:
                order = [(k, j) for k in range(n_rows) for j in range(n_chunks_per_row)]
            last_p2_dma = None
            for k, j in order:
                wst2 = wstage.tile(
                    [128, chunk], F32, tag=f"{tag}st2", name=f"{tag}st2", bufs=4
                )
                last_p2_dma = nc.sync.dma_start(
                    wst2[:], wt_d[k * 128 : (k + 1) * 128, j * chunk : (j + 1) * chunk]
                )
                nc.vector.tensor_scalar(
                    wst2[:], wst2[:], inv_s[:], C_RNE, op0=ALU.mult, op1=ALU.add
                )
                nc.vector.tensor_scalar(
                    dst_tiles[k][:, j * chunk : (j + 1) * chunk],
                    wst2[:], C_RNE, None, op0=ALU.subtract,
                )
            return scale, inv_s, macc_last

        w1q = [wq.tile([128, H], BF16, tag=f"w1q{d}", name=f"w1q{d}") for d in range(KD)]
        w2q = [wq.tile([128, D], BF16, tag=f"w2q{t}", name=f"w2q{t}") for t in range(KH)]

        def load_x_block(m0, m_blk):
            """DMA one x block (SWDGE — keeps it off the bulk HWDGE stream)
            and split into bf16 hi (+ lo)."""
            xh, xl = [], []
            for d in range(KD):
                xs_ = xstage.tile([128, m_blk], F32, tag=f"xs{d}", name=f"xs{d}")
                nc.gpsimd.dma_start(xs_[:], xt_d[d * 128 : (d + 1) * 128, m0 : m0 + m_blk])
                xh_ = xsplit.tile([128, m_blk], BF16, tag=f"xh{d}", name=f"xh{d}")
                nc.vector.tensor_copy(xh_[:], xs_[:])
                xh.append(xh_)
                if use_split:
                    xl_ = xsplit.tile([128, m_blk], BF16, tag=f"xl{d}", name=f"xl{d}")
                    nc.vector.tensor_tensor(xl_[:], xs_[:], xh_[:], op=ALU.subtract)
                    xl.append(xl_)
            return xh, xl

        s1, inv_s1, q1_macc = quantize(w1t_d, KD, H, 768, w1q, "q1",
                                       pass2_j_major=True)
        x0 = load_x_block(0, M_BLOCKS[0])
        # w2's bulk pass-1 stream must not steal DMA bandwidth from w1's
        # (which gates everything); w1 pass-2 self-throttles via its 3
        # staging slots, so only w2 needs an explicit gate.
        s2, _, _ = quantize(w2t_d, KH, D, D, w2q, "q2",
                            gate_pass1_on=q1_macc.ins)

        # b1' = b1 / s1   (per-partition column layout [128, KH])
        b1s = const.tile([128, KH], F32, tag="b1s")
        nc.vector.tensor_scalar(b1s[:], b1_pack[:], inv_s1[:], None, op0=ALU.mult)
        # c = s1 * s2  (final output scale), already per-partition [128,1]
        cscale = scal.tile([128, 1], F32, tag="cscale")
        nc.vector.tensor_tensor(cscale[:], s1[:], s2[:], op=ALU.mult)

        # ---------- main pipeline over row blocks ----------
        def fc1_block(m_blk, xh, xl):
            """fc1: hT[t] = relu(contract_d(w1q, xT) + b1')  (hidden on
            partitions).  Returns (hh, hl) bf16 hi/lo tiles."""
            hh, hl = [], []
            for t in range(KH):
                ps = ps1.tile([128, m_blk], F32, tag="ps1", name="ps1")
                total = KD * (2 if use_split else 1)
                cnt = 0
                for d in range(KD):
                    lhs = w1q[d][:, t * 128 : (t + 1) * 128]
                    nc.tensor.matmul(
                        ps[:], lhs, xh[d][:], start=(cnt == 0), stop=(cnt == total - 1)
                    )
                    cnt += 1
                    if use_split:
                        nc.tensor.matmul(
                            ps[:], lhs, xl[d][:], start=False, stop=(cnt == total - 1)
                        )
                        cnt += 1
                hf = hf32p.tile([128, m_blk], F32, tag="hf", name="hf")
                nc.scalar.activation(hf[:], ps[:], ACTF.Relu, bias=b1s[:, t : t + 1])
                hh_ = hpool.tile([128, m_blk], BF16, tag=f"hh{t}", name=f"hh{t}")
                nc.scalar.activation(hh_[:], hf[:], ACTF.Copy)
                hh.append(hh_)
                if use_split:
                    hl_ = hpool.tile([128, m_blk], BF16, tag=f"hl{t}", name=f"hl{t}")
                    nc.vector.tensor_tensor(hl_[:], hf[:], hh_[:], op=ALU.subtract)
                    hl.append(hl_)
            return hh, hl

        def fc2_block(m0, m_blk, hh, hl):
            """fc2: out[m, :] = c * contract_h(hT, w2q) + b2"""
            parts = [hh, hl] if use_split else [hh]
            off = 0
            while off < m_blk:
                msz = min(128, m_blk - off)
                ms0 = off
                off += msz
                halves = []
                for h0, hw in ((0, 384), (384, 384)):
                    ps_ = ps2.tile([128, 384], F32, tag=f"ps2_{h0}", name=f"ps2_{h0}")
                    total = KH * len(parts)
                    cnt = 0
                    for t in range(KH):
                        for part in parts:
                            nc.tensor.matmul(
                                ps_[:msz, :],
                                part[t][:, ms0 : ms0 + msz],
                                w2q[t][:, h0 : h0 + hw],
                                start=(cnt == 0),
                                stop=(cnt == total - 1),
                            )
                            cnt += 1
                    halves.append((h0, hw, ps_))
                ot = opool.tile([128, D], F32, tag="ot", name="ot")
                for h0, hw, ps_ in halves:
                    nc.vector.tensor_scalar(
                        ot[:msz, h0 : h0 + hw], ps_[:msz, :], cscale[:msz, :], None,
                        op0=ALU.mult,
                    )
                nc.vector.tensor_tensor(
                    ot[:msz, :], ot[:msz, :], b2_bc[:msz, :], op=ALU.add
                )
                row = m0 + ms0
                nc.sync.dma_start(out_d[row : row + msz, :], ot[:msz, :])

        # Interleave: fc1 of block k+1 is emitted before fc2 of block k so
        # the PE always has fc1 work while fc2's inputs (w2q early on, h
        # tiles later) are still being produced.  h tiles are double
        # buffered (hpool bufs=2) to allow this.
        starts = []
        o = 0
        for mb in M_BLOCKS:
            starts.append(o)
            o += mb
        prev = None
        for blk, m_blk in enumerate(M_BLOCKS):
            xh, xl = x0 if blk == 0 else load_x_block(starts[blk], m_blk)
            hh, hl = fc1_block(m_blk, xh, xl)
            if prev is not None:
                fc2_block(starts[blk - 1], M_BLOCKS[blk - 1], *prev)
            prev = (hh, hl)
        fc2_block(starts[-1], M_BLOCKS[-1], *prev)

    if walrus_fixups:
        _split_oversized_waits(nc)
    return nc


_PROGRAM_CACHE = {}


def _get_program(qmax: float, use_split: bool = USE_SPLIT):
    key = (qmax, use_split)
    if key not in _PROGRAM_CACHE:
        _PROGRAM_CACHE[key] = build_program(qmax, use_split)
    return _PROGRAM_CACHE[key]


def kernel(x, w1, b1, w2, b2, bits):
    qmax = float(2.0 ** (int(bits) - 1) - 1.0)
    nc = _get_program(qmax)

    x = np.ascontiguousarray(np.asarray(x, dtype=np.float32)).reshape(M_TOTAL, D)
    w1t = np.ascontiguousarray(np.asarray(w1, dtype=np.float32).T)   # [768, 3072]
    w2t = np.ascontiguousarray(np.asarray(w2, dtype=np.float32).T)   # [3072, 768]
    b1h = np.ascontiguousarray(
        np.asarray(b1, dtype=np.float32).reshape(KH, 128).T
    )  # [128, KH]
    b2h = np.ascontiguousarray(
        np.broadcast_to(np.asarray(b2, dtype=np.float32).reshape(1, D), (128, D))
    )  # [128, D]
    xt_full = np.ascontiguousarray(x.T)                              # [768, 12544]

    in_maps = []
    for c in range(N_CORES):
        xt_c = np.ascontiguousarray(xt_full[:, c * M_SHARD : (c + 1) * M_SHARD])
        in_maps.append(
            {"xt": xt_c, "w1t": w1t, "w2t": w2t, "b1": b1h, "b2": b2h}
        )

    res = bass_utils.run_bass_kernel_spmd(nc, in_maps, core_ids=list(range(N_CORES)))
    out = np.concatenate(
        [res.results[c]["out"][:M_SHARD] for c in range(N_CORES)], axis=0
    )
    return np.ascontiguousarray(out.reshape(B, S, D))


# revision 57
# speedup vs baseline: 1.1541x; 1.0409x over previous
"""Trainium2 Bass kernel for nn_Dyanmic_Q_MLP (fake-quant MLP).

Computation (reference):
    w1q = fake_quant(w1, 8); w2q = fake_quant(w2, 8)       # per-tensor symmetric
    h   = relu(x @ w1q.T + b1)                             # [B,S,3072]
    out = h @ w2q.T + b2                                   # [B,S,768]

Strategy:
  * Data-parallel over the flattened (B*S)=12544 rows across 8 NeuronCores
    (1568 rows/core, zero-padded to 1664 = 13*128). Weights replicated.
    No collectives.
  * Host side only reshapes/transposes/shards (layout, no math):
      xt  = x.T slice per core   [768, 1664]
      w1t = w1.T                 [768, 3072]
      w2t = w2.T                 [3072, 768]
  * On-device fake-quant: global abs-max (DVE reduce + GPSIMD partition
    all-reduce), scale = max/qmax, integer-valued weights q = round(w/scale)
    via the +-1.5*2^23 RNE trick.  q in [-127,127] is EXACTLY representable
    in bf16, so matmuls run on the bf16 PE path with no weight error; the
    scale is folded into the epilogues (relu(s1*z+b1) = s1*relu(z+b1/s1)).
  * Activations are split hi/lo into two bf16 operands (x = hi + lo), giving
    ~fp32-accurate matmuls at 2x bf16 cost (vs 4x for native fp32 path).
"""

import sys

for _p in ("/opt/trn_rl_repo", "/root/.axon_site/_ro/trn_rl_repo"):
    if _p not in sys.path:
        sys.path.insert(0, _p)

from contextlib import ExitStack

import numpy as np

import concourse.bass as bass
import concourse.mybir as mybir
import concourse.tile as tile
from concourse import bass_utils
from concourse.tile_rust import add_dep_helper

N_CORES = 8
B, S, D, H = 64, 196, 768, 3072
M_TOTAL = B * S            # 12544
M_SHARD = M_TOTAL // N_CORES   # 1568
M_PAD = M_SHARD            # no padding
# 6 blocks of 256 + a 32-row tail: 13 fc2 row-subtiles total (N-bound cost),
# small blocks let fc1 of block k+1 overlap fc2 of block k (h double-buffer)
M_BLOCKS = [256] * 6 + [32]
KD = D // 128              # 6
KH = H // 128              # 24
C_RNE = 12582912.0         # 1.5 * 2**23: (v + C) - C == round-to-nearest-even(v)
USE_SPLIT = True           # hi/lo bf16 split of activations (pseudo-fp32)

F32 = mybir.dt.float32
BF16 = mybir.dt.bfloat16
ALU = mybir.AluOpType
ACTF = mybir.ActivationFunctionType


def _split_oversized_waits(nc, max_waits=1):
    """The walrus build in this container accepts only one sync-wait per
    instruction.  Hoist excess on_wait entries onto inserted same-engine
    NoOp instructions placed just before (queue-order preserves semantics;
    a NoOp-with-wait stalls the queue without flushing the engine pipe)."""
    for f in nc.m.functions:
        for b in f.blocks:
            new_list, changed, ctr = [], False, 0
            for i in b.instructions:
                si = i.sync_info
                w = list(si.on_wait) if si is not None else []
                if len(w) > max_waits:
                    extra, keep = w[:-max_waits], w[-max_waits:]
                    for ci in range(0, len(extra), max_waits):
                        ctr += 1
                        d = mybir.InstNoOp(
                            name=f"{i.name}-wsplit{ctr}",
                            engine=i.engine,
                        )
                        d.sync_info = mybir.SyncInfo(
                            on_update=[], on_wait=extra[ci : ci + max_waits]
                        )
                        new_list.append(d)
                    si.on_wait = keep
                    changed = True
                new_list.append(i)
            if changed:
                b.instructions = new_list


def build_program(qmax: float, use_split: bool = USE_SPLIT, walrus_fixups: bool = True):
    """Build the per-core Bass program (same NEFF on all 8 cores).

    walrus_fixups: apply _split_oversized_waits (needed for the walrus
    compile; CoreSim chokes on the inserted drains, so sim callers pass
    False)."""
    nc = bass.Bass("TRN2", target_bir_lowering=False, debug=False)

    xt_d = nc.dram_tensor("xt", (D, M_PAD), F32, kind="ExternalInput").ap()
    w1t_d = nc.dram_tensor("w1t", (D, H), F32, kind="ExternalInput").ap()
    w2t_d = nc.dram_tensor("w2t", (H, D), F32, kind="ExternalInput").ap()
    # b1 comes host-side pre-packed as [128, KH]: column t holds
    # b1[t*128:(t+1)*128]; b2 likewise as [128, KD].
    b1_d = nc.dram_tensor("b1", (128, KH), F32, kind="ExternalInput").ap()
    b2_d = nc.dram_tensor("b2", (128, KD), F32, kind="ExternalInput").ap()
    id_d = nc.dram_tensor("ident", (128, 128), F32, kind="ExternalInput").ap()
    # fc2 computes out.T (d on partitions); the host untransposes.
    out_d = nc.dram_tensor("outT", (D, M_PAD), F32, kind="ExternalOutput").ap()

    with tile.TileContext(nc) as tc, ExitStack() as ctx:
        const = ctx.enter_context(tc.tile_pool(name="const", bufs=1))
        wq = ctx.enter_context(tc.tile_pool(name="wq", bufs=1))
        wstage = ctx.enter_context(tc.tile_pool(name="wstage", bufs=2))
        xstage = ctx.enter_context(tc.tile_pool(name="xstage", bufs=2))
        xsplit = ctx.enter_context(tc.tile_pool(name="xsplit", bufs=2))
        hpool = ctx.enter_context(tc.tile_pool(name="hpool", bufs=2))
        hf32p = ctx.enter_context(tc.tile_pool(name="hf32p", bufs=2))
        opool = ctx.enter_context(tc.tile_pool(name="opool", bufs=2))
        scal = ctx.enter_context(tc.tile_pool(name="scal", bufs=1))
        ps1 = ctx.enter_context(tc.tile_pool(name="ps1", bufs=4, space="PSUM"))
        ps2 = ctx.enter_context(tc.tile_pool(name="ps2", bufs=2, space="PSUM"))
        dram = ctx.enter_context(tc.tile_pool(name="dram", bufs=1, space="DRAM"))

        # ---------- setup: biases (already laid out by the host) ----------
        b1_pack = const.tile([128, KH], F32, tag="b1pack")
        nc.sync.dma_start(b1_pack[:], b1_d[:])
        b2_pack = const.tile([128, KD], F32, tag="b2pack")
        nc.sync.dma_start(b2_pack[:], b2_d[:])
        ident = const.tile([128, 128], F32, tag="ident")
        nc.sync.dma_start(ident[:], id_d[:])
        ones_row = const.tile([1, 128], F32, tag="ones_row")
        nc.vector.memset(ones_row[:], 1.0)

        # ---------- fake-quant of weights ----------
        def quantize(wt_d, n_rows, free_dim, chunk, dst_tiles, tag,
                     pass2_j_major=False, gate_pass1_on=None,
                     pass2_free_chunks=0, pass1_chunk=None):
            """Two passes over wt_d ([n_rows*128, free_dim] DRAM, row-major):
            pass1 computes the global abs-max, pass2 re-loads and writes
            round(w/scale) as bf16 integers into dst_tiles[k][:, :].
            pass2_j_major orders pass-2 chunks column-block-major so the
            first matmuls (which need the leading columns of EVERY row
            tile) unblock as early as possible.
            Returns (scale[1,1], inv_scale[128,1]) tiles."""
            p1c = pass1_chunk or chunk
            n_chunks_per_row = free_dim // p1c
            macc = scal.tile([128, 1], F32, tag=f"{tag}macc")
            first = True
            macc_last = None
            for k in range(n_rows):
                for j in range(n_chunks_per_row):
                    wst = wstage.tile([128, p1c], F32, tag=f"{tag}st", bufs=4)
                    dma = nc.sync.dma_start(
                        wst[:], wt_d[k * 128 : (k + 1) * 128, j * p1c : (j + 1) * p1c]
                    )
                    if gate_pass1_on is not None:
                        add_dep_helper(dma.ins, gate_pass1_on,
                                       reason="serialize bulk weight DMA streams")
                    mk = scal.tile([128, 1], F32, tag=f"{tag}mk", bufs=2)
                    nc.vector.tensor_reduce(
                        mk[:], wst[:], axis=mybir.AxisListType.X,
                        op=ALU.max, apply_absolute_value=True,
                    )
                    if first:
                        macc_last = nc.vector.tensor_copy(macc[:], mk[:])
                        first = False
                    else:
                        macc_last = nc.vector.tensor_tensor(
                            macc[:], macc[:], mk[:], op=ALU.max
                        )
            # cross-partition max, replicated to every partition via exact
            # PE transposes (f32 transpose mode moves raw values):
            #   macc[128,1] -T-> [1,128] -reduce-> [1,1] -x ones-> [1,128]
            #   -T-> [128,1]
            rps = ps2.tile([1, 128], F32, tag="redT", name=f"{tag}rps", bufs=1)
            nc.tensor.transpose(rps[:], macc[:], ident[:])
            mrow = scal.tile([1, 128], F32, tag=f"{tag}mrow", name=f"{tag}mrow")
            nc.vector.tensor_copy(mrow[:], rps[:])
            g11 = scal.tile([1, 1], F32, tag=f"{tag}g11", name=f"{tag}g11")
            nc.vector.tensor_reduce(g11[:], mrow[:], axis=mybir.AxisListType.X, op=ALU.max)
            grow = scal.tile([1, 128], F32, tag=f"{tag}grow", name=f"{tag}grow")
            nc.vector.tensor_scalar(grow[:], ones_row[:], g11[:], None, op0=ALU.mult)
            gps = ps2.tile([128, 1], F32, tag="redT2", name=f"{tag}gps", bufs=1)
            nc.tensor.transpose(gps[:], grow[:], ident[:1, :1])
            gmax = scal.tile([128, 1], F32, tag=f"{tag}gmax", name=f"{tag}gmax")
            nc.vector.tensor_copy(gmax[:], gps[:])
            # walrus rejects ALU divide in tensor_scalar; mult by 1/qmax
            # differs from max/qmax by <=1 ulp (negligible: it only shifts
            # the global output scale by ~1e-7 relative).
            scale = scal.tile([128, 1], F32, tag=f"{tag}scale", name=f"{tag}scale")
            nc.vector.tensor_scalar(scale[:], gmax[:], 1.0 / float(qmax), None, op0=ALU.mult)
            inv_s = scal.tile([128, 1], F32, tag=f"{tag}inv", name=f"{tag}inv")
            nc.vector.reciprocal(inv_s[:], scale[:])
            # pass 2: round(w * inv_s) -> bf16 (exact integers)
            n_j2 = free_dim // chunk
            if pass2_j_major:
                order = [(k, j) for j in range(n_j2) for k in range(n_rows)]
            else:
                order = [(k, j) for k in range(n_rows) for j in range(n_j2)]
            last_p2_dma = None
            for ci, (k, j) in enumerate(order):
                wst2 = wstage.tile(
                    [128, chunk], F32, tag=f"{tag}st2", name=f"{tag}st2", bufs=4
                )
                last_p2_dma = nc.sync.dma_start(
                    wst2[:], wt_d[k * 128 : (k + 1) * 128, j * chunk : (j + 1) * chunk]
                )
                if ci >= pass2_free_chunks:
                    # later chunks must not steal DMA bandwidth from the
                    # pass-1 max stream (which gates everything)
                    add_dep_helper(last_p2_dma.ins, macc_last.ins,
                                   reason="pass2 bulk re-DMA after pass1 max")
                nc.vector.tensor_scalar(
                    wst2[:], wst2[:], inv_s[:], C_RNE, op0=ALU.mult, op1=ALU.add
                )
                nc.vector.tensor_scalar(
                    dst_tiles[k][:, j * chunk : (j + 1) * chunk],
                    wst2[:], C_RNE, None, op0=ALU.subtract,
                )
            return scale, inv_s, macc_last

        w1q = [wq.tile([128, H], BF16, tag=f"w1q{d}", name=f"w1q{d}") for d in range(KD)]
        w2q = [wq.tile([128, D], BF16, tag=f"w2q{t}", name=f"w2q{t}") for t in range(KH)]

        def load_x_block(m0, m_blk):
            """DMA one x block (SWDGE — keeps it off the bulk HWDGE stream)
            and split into bf16 hi (+ lo)."""
            xh, xl = [], []
            for d in range(KD):
                xs_ = xstage.tile([128, m_blk], F32, tag=f"xs{d}", name=f"xs{d}")
                nc.gpsimd.dma_start(xs_[:], xt_d[d * 128 : (d + 1) * 128, m0 : m0 + m_blk])
                xh_ = xsplit.tile([128, m_blk], BF16, tag=f"xh{d}", name=f"xh{d}")
                nc.vector.tensor_copy(xh_[:], xs_[:])
                xh.append(xh_)
                if use_split:
                    xl_ = xsplit.tile([128, m_blk], BF16, tag=f"xl{d}", name=f"xl{d}")
                    nc.vector.tensor_tensor(xl_[:], xs_[:], xh_[:], op=ALU.subtract)
                    xl.append(xl_)
            return xh, xl

        s1, inv_s1, q1_macc = quantize(w1t_d, KD, H, 768, w1q, "q1",
                                       pass2_j_major=True, pass2_free_chunks=6)
        x0 = load_x_block(0, M_BLOCKS[0])
        # w2's bulk pass-1 stream must not steal DMA bandwidth from w1's
        # (which gates everything).
        s2, _, _ = quantize(w2t_d, KH, D, D, w2q, "q2",
                            gate_pass1_on=q1_macc.ins, pass2_free_chunks=KH)

        # b1' = b1 / s1   (per-partition column layout [128, KH])
        b1s = const.tile([128, KH], F32, tag="b1s")
        nc.vector.tensor_scalar(b1s[:], b1_pack[:], inv_s1[:], None, op0=ALU.mult)
        # c = s1 * s2  (final output scale), already per-partition [128,1]
        cscale = scal.tile([128, 1], F32, tag="cscale")
        nc.vector.tensor_tensor(cscale[:], s1[:], s2[:], op=ALU.mult)

        # ---------- main pipeline over row blocks ----------
        def fc1_block(m_blk, xh, xl):
            """fc1: hT[t] = relu(contract_d(w1q, xT) + b1')  (hidden on
            partitions).  Returns (hh, hl) bf16 hi/lo tiles."""
            hh, hl = [], []
            for t in range(KH):
                ps = ps1.tile([128, m_blk], F32, tag="ps1", name="ps1")
                total = KD * (2 if use_split else 1)
                cnt = 0
                for d in range(KD):
                    lhs = w1q[d][:, t * 128 : (t + 1) * 128]
                    nc.tensor.matmul(
                        ps[:], lhs, xh[d][:], start=(cnt == 0), stop=(cnt == total - 1)
                    )
                    cnt += 1
                    if use_split:
                        nc.tensor.matmul(
                            ps[:], lhs, xl[d][:], start=False, stop=(cnt == total - 1)
                        )
                        cnt += 1
                hf = hf32p.tile([128, m_blk], F32, tag="hf", name="hf")
                nc.scalar.activation(hf[:], ps[:], ACTF.Relu, bias=b1s[:, t : t + 1])
                hh_ = hpool.tile([128, m_blk], BF16, tag=f"hh{t}", name=f"hh{t}")
                nc.scalar.activation(hh_[:], hf[:], ACTF.Copy)
                hh.append(hh_)
                if use_split:
                    hl_ = hpool.tile([128, m_blk], BF16, tag=f"hl{t}", name=f"hl{t}")
                    nc.vector.tensor_tensor(hl_[:], hf[:], hh_[:], op=ALU.subtract)
                    hl.append(hl_)
            return hh, hl

        def fc2_block(m0, m_blk, hh, hl):
            """fc2 (transposed): outT[d, m] = c * contract_h(w2q, hT) + b2.
            w2q is the stationary operand, hT the moving one, so the row
            count only enters as streaming cycles (no N-penalty for the
            32-row tail) and the epilogue fuses scale+bias in one ACT op."""
            parts = [hh, hl] if use_split else [hh]
            for dt in range(KD):
                ps_ = ps2.tile([128, m_blk], F32, tag="ps2", name="ps2")
                total = KH * len(parts)
                cnt = 0
                for t in range(KH):
                    lhs = w2q[t][:, dt * 128 : (dt + 1) * 128]
                    for part in parts:
                        nc.tensor.matmul(
                            ps_[:], lhs, part[t][:],
                            start=(cnt == 0), stop=(cnt == total - 1),
                        )
                        cnt += 1
                ot = opool.tile([128, m_blk], F32, tag="ot", name="ot")
                # out = Identity(psum * c + b2)  — one ACT op
                nc.scalar.activation(
                    ot[:], ps_[:], ACTF.Identity,
                    bias=b2_pack[:, dt : dt + 1], scale=cscale[:],
                )
                nc.sync.dma_start(
                    out_d[dt * 128 : (dt + 1) * 128, m0 : m0 + m_blk], ot[:]
                )

        # Interleave: fc1 of block k+1 is emitted before fc2 of block k so
        # the PE always has fc1 work while fc2's inputs (w2q early on, h
        # tiles later) are still being produced.  h tiles are double
        # buffered (hpool bufs=2) to allow this.
        starts = []
        o = 0
        for mb in M_BLOCKS:
            starts.append(o)
            o += mb
        prev = None
        for blk, m_blk in enumerate(M_BLOCKS):
            xh, xl = x0 if blk == 0 else load_x_block(starts[blk], m_blk)
            hh, hl = fc1_block(m_blk, xh, xl)
            if prev is not None:
                fc2_block(starts[blk - 1], M_BLOCKS[blk - 1], *prev)
            prev = (hh, hl)
        fc2_block(starts[-1], M_BLOCKS[-1], *prev)

    if walrus_fixups:
        _split_oversized_waits(nc)
    return nc


_PROGRAM_CACHE = {}


def _get_program(qmax: float, use_split: bool = USE_SPLIT):
    key = (qmax, use_split)
    if key not in _PROGRAM_CACHE:
        _PROGRAM_CACHE[key] = build_program(qmax, use_split)
    return _PROGRAM_CACHE[key]


def kernel(x, w1, b1, w2, b2, bits):
    qmax = float(2.0 ** (int(bits) - 1) - 1.0)
    nc = _get_program(qmax)

    x = np.ascontiguousarray(np.asarray(x, dtype=np.float32)).reshape(M_TOTAL, D)
    w1t = np.ascontiguousarray(np.asarray(w1, dtype=np.float32).T)   # [768, 3072]
    w2t = np.ascontiguousarray(np.asarray(w2, dtype=np.float32).T)   # [3072, 768]
    b1h = np.ascontiguousarray(
        np.asarray(b1, dtype=np.float32).reshape(KH, 128).T
    )  # [128, KH]
    b2h = np.ascontiguousarray(
        np.asarray(b2, dtype=np.float32).reshape(KD, 128).T
    )  # [128, KD]
    xt_full = np.ascontiguousarray(x.T)                              # [768, 12544]

    ident = np.eye(128, dtype=np.float32)
    in_maps = []
    for c in range(N_CORES):
        xt_c = np.ascontiguousarray(xt_full[:, c * M_SHARD : (c + 1) * M_SHARD])
        in_maps.append(
            {"xt": xt_c, "w1t": w1t, "w2t": w2t, "b1": b1h, "b2": b2h,
             "ident": ident}
        )

    res = bass_utils.run_bass_kernel_spmd(nc, in_maps, core_ids=list(range(N_CORES)))
    out = np.concatenate(
        [res.results[c]["outT"].T for c in range(N_CORES)], axis=0
    )
    return np.ascontiguousarray(out.reshape(B, S, D))


# revision 64
# speedup vs baseline: 1.1589x; 1.0041x over previous
"""Trainium2 Bass kernel for nn_Dyanmic_Q_MLP (fake-quant MLP).

Computation (reference):
    w1q = fake_quant(w1, 8); w2q = fake_quant(w2, 8)       # per-tensor symmetric
    h   = relu(x @ w1q.T + b1)                             # [B,S,3072]
    out = h @ w2q.T + b2                                   # [B,S,768]

Strategy:
  * Data-parallel over the flattened (B*S)=12544 rows across 8 NeuronCores
    (1568 rows/core, zero-padded to 1664 = 13*128). Weights replicated.
    No collectives.
  * Host side only reshapes/transposes/shards (layout, no math):
      xt  = x.T slice per core   [768, 1664]
      w1t = w1.T                 [768, 3072]
      w2t = w2.T                 [3072, 768]
  * On-device fake-quant: global abs-max (DVE reduce + GPSIMD partition
    all-reduce), scale = max/qmax, integer-valued weights q = round(w/scale)
    via the +-1.5*2^23 RNE trick.  q in [-127,127] is EXACTLY representable
    in bf16, so matmuls run on the bf16 PE path with no weight error; the
    scale is folded into the epilogues (relu(s1*z+b1) = s1*relu(z+b1/s1)).
  * Activations are split hi/lo into two bf16 operands (x = hi + lo), giving
    ~fp32-accurate matmuls at 2x bf16 cost (vs 4x for native fp32 path).
"""

import sys

for _p in ("/opt/trn_rl_repo", "/root/.axon_site/_ro/trn_rl_repo"):
    if _p not in sys.path:
        sys.path.insert(0, _p)

from contextlib import ExitStack

import numpy as np

import concourse.bass as bass
import concourse.mybir as mybir
import concourse.tile as tile
from concourse import bass_utils
from concourse.tile_rust import add_dep_helper

N_CORES = 8
B, S, D, H = 64, 196, 768, 3072
M_TOTAL = B * S            # 12544
M_SHARD = M_TOTAL // N_CORES   # 1568
M_PAD = M_SHARD            # no padding
# 6 blocks of 256 + a 32-row tail: 13 fc2 row-subtiles total (N-bound cost),
# small blocks let fc1 of block k+1 overlap fc2 of block k (h double-buffer)
M_BLOCKS = [256] * 6 + [32]
KD = D // 128              # 6
KH = H // 128              # 24
C_RNE = 12582912.0         # 1.5 * 2**23: (v + C) - C == round-to-nearest-even(v)
USE_SPLIT = True           # hi/lo bf16 split of activations (pseudo-fp32)

F32 = mybir.dt.float32
BF16 = mybir.dt.bfloat16
ALU = mybir.AluOpType
ACTF = mybir.ActivationFunctionType


def _split_oversized_waits(nc, max_waits=1):
    """The walrus build in this container accepts only one sync-wait per
    instruction.  Hoist excess on_wait entries onto inserted same-engine
    NoOp instructions placed just before (queue-order preserves semantics;
    a NoOp-with-wait stalls the queue without flushing the engine pipe)."""
    for f in nc.m.functions:
        for b in f.blocks:
            new_list, changed, ctr = [], False, 0
            for i in b.instructions:
                si = i.sync_info
                w = list(si.on_wait) if si is not None else []
                if len(w) > max_waits:
                    extra, keep = w[:-max_waits], w[-max_waits:]
                    for ci in range(0, len(extra), max_waits):
                        ctr += 1
                        d = mybir.InstNoOp(
                            name=f"{i.name}-wsplit{ctr}",
                            engine=i.engine,
                        )
                        d.sync_info = mybir.SyncInfo(
                            on_update=[], on_wait=extra[ci : ci + max_waits]
                        )
                        new_list.append(d)
                    si.on_wait = keep
                    changed = True
                new_list.append(i)
            if changed:
                b.instructions = new_list


def build_program(qmax: float, use_split: bool = USE_SPLIT, walrus_fixups: bool = True):
    """Build the per-core Bass program (same NEFF on all 8 cores).

    walrus_fixups: apply _split_oversized_waits (needed for the walrus
    compile; CoreSim chokes on the inserted drains, so sim callers pass
    False)."""
    nc = bass.Bass("TRN2", target_bir_lowering=False, debug=False)

    xt_d = nc.dram_tensor("xt", (D, M_PAD), F32, kind="ExternalInput").ap()
    w1t_d = nc.dram_tensor("w1t", (D, H), F32, kind="ExternalInput").ap()
    w2t_d = nc.dram_tensor("w2t", (H, D), F32, kind="ExternalInput").ap()
    # b1 comes host-side pre-packed as [128, KH]: column t holds
    # b1[t*128:(t+1)*128]; b2 likewise as [128, KD].
    b1_d = nc.dram_tensor("b1", (128, KH), F32, kind="ExternalInput").ap()
    b2_d = nc.dram_tensor("b2", (128, KD), F32, kind="ExternalInput").ap()
    id_d = nc.dram_tensor("ident", (128, 128), F32, kind="ExternalInput").ap()
    # fc2 computes out.T (d on partitions); the host untransposes.
    out_d = nc.dram_tensor("outT", (D, M_PAD), F32, kind="ExternalOutput").ap()

    with tile.TileContext(nc) as tc, ExitStack() as ctx:
        const = ctx.enter_context(tc.tile_pool(name="const", bufs=1))
        wq = ctx.enter_context(tc.tile_pool(name="wq", bufs=1))
        wstage = ctx.enter_context(tc.tile_pool(name="wstage", bufs=2))
        xstage = ctx.enter_context(tc.tile_pool(name="xstage", bufs=2))
        xsplit = ctx.enter_context(tc.tile_pool(name="xsplit", bufs=2))
        hpool = ctx.enter_context(tc.tile_pool(name="hpool", bufs=2))
        hf32p = ctx.enter_context(tc.tile_pool(name="hf32p", bufs=2))
        opool = ctx.enter_context(tc.tile_pool(name="opool", bufs=2))
        scal = ctx.enter_context(tc.tile_pool(name="scal", bufs=1))
        ps1 = ctx.enter_context(tc.tile_pool(name="ps1", bufs=4, space="PSUM"))
        ps2 = ctx.enter_context(tc.tile_pool(name="ps2", bufs=3, space="PSUM"))
        dram = ctx.enter_context(tc.tile_pool(name="dram", bufs=1, space="DRAM"))

        # ---------- setup: biases (already laid out by the host) ----------
        b1_pack = const.tile([128, KH], F32, tag="b1pack")
        nc.sync.dma_start(b1_pack[:], b1_d[:])
        b2_pack = const.tile([128, KD], F32, tag="b2pack")
        nc.sync.dma_start(b2_pack[:], b2_d[:])
        ident = const.tile([128, 128], F32, tag="ident")
        nc.sync.dma_start(ident[:], id_d[:])
        ones_row = const.tile([1, 128], F32, tag="ones_row")
        nc.vector.memset(ones_row[:], 1.0)

        # ---------- fake-quant of weights ----------
        def quantize(wt_d, n_rows, free_dim, chunk, dst_tiles, tag,
                     pass2_j_major=False, gate_pass1_on=None,
                     pass2_free_chunks=0, pass1_chunk=None):
            """Two passes over wt_d ([n_rows*128, free_dim] DRAM, row-major):
            pass1 computes the global abs-max, pass2 re-loads and writes
            round(w/scale) as bf16 integers into dst_tiles[k][:, :].
            pass2_j_major orders pass-2 chunks column-block-major so the
            first matmuls (which need the leading columns of EVERY row
            tile) unblock as early as possible.
            Returns (scale[1,1], inv_scale[128,1]) tiles."""
            p1c = pass1_chunk or chunk
            n_chunks_per_row = free_dim // p1c
            macc = scal.tile([128, 1], F32, tag=f"{tag}macc")
            first = True
            macc_last = None
            for k in range(n_rows):
                for j in range(n_chunks_per_row):
                    wst = wstage.tile([128, p1c], F32, tag=f"{tag}st", bufs=4)
                    dma = nc.sync.dma_start(
                        wst[:], wt_d[k * 128 : (k + 1) * 128, j * p1c : (j + 1) * p1c]
                    )
                    if gate_pass1_on is not None:
                        add_dep_helper(dma.ins, gate_pass1_on,
                                       reason="serialize bulk weight DMA streams")
                    mk = scal.tile([128, 1], F32, tag=f"{tag}mk", bufs=2)
                    nc.vector.tensor_reduce(
                        mk[:], wst[:], axis=mybir.AxisListType.X,
                        op=ALU.max, apply_absolute_value=True,
                    )
                    if first:
                        macc_last = nc.vector.tensor_copy(macc[:], mk[:])
                        first = False
                    else:
                        macc_last = nc.vector.tensor_tensor(
                            macc[:], macc[:], mk[:], op=ALU.max
                        )
            # cross-partition max, replicated to every partition via exact
            # PE transposes (f32 transpose mode moves raw values):
            #   macc[128,1] -T-> [1,128] -reduce-> [1,1] -x ones-> [1,128]
            #   -T-> [128,1]
            rps = ps2.tile([1, 128], F32, tag="redT", name=f"{tag}rps", bufs=1)
            nc.tensor.transpose(rps[:], macc[:], ident[:])
            mrow = scal.tile([1, 128], F32, tag=f"{tag}mrow", name=f"{tag}mrow")
            nc.vector.tensor_copy(mrow[:], rps[:])
            g11 = scal.tile([1, 1], F32, tag=f"{tag}g11", name=f"{tag}g11")
            nc.vector.tensor_reduce(g11[:], mrow[:], axis=mybir.AxisListType.X, op=ALU.max)
            grow = scal.tile([1, 128], F32, tag=f"{tag}grow", name=f"{tag}grow")
            nc.vector.tensor_scalar(grow[:], ones_row[:], g11[:], None, op0=ALU.mult)
            gps = ps2.tile([128, 1], F32, tag="redT", name=f"{tag}gps", bufs=1)
            nc.tensor.transpose(gps[:], grow[:], ident[:1, :1])
            gmax = scal.tile([128, 1], F32, tag=f"{tag}gmax", name=f"{tag}gmax")
            nc.vector.tensor_copy(gmax[:], gps[:])
            # walrus rejects ALU divide in tensor_scalar; mult by 1/qmax
            # differs from max/qmax by <=1 ulp (negligible: it only shifts
            # the global output scale by ~1e-7 relative).
            scale = scal.tile([128, 1], F32, tag=f"{tag}scale", name=f"{tag}scale")
            nc.vector.tensor_scalar(scale[:], gmax[:], 1.0 / float(qmax), None, op0=ALU.mult)
            inv_s = scal.tile([128, 1], F32, tag=f"{tag}inv", name=f"{tag}inv")
            nc.vector.reciprocal(inv_s[:], scale[:])
            # pass 2: round(w * inv_s) -> bf16 (exact integers)
            n_j2 = free_dim // chunk
            if pass2_j_major:
                order = [(k, j) for j in range(n_j2) for k in range(n_rows)]
            else:
                order = [(k, j) for k in range(n_rows) for j in range(n_j2)]
            last_p2_dma = None
            for ci, (k, j) in enumerate(order):
                wst2 = wstage.tile(
                    [128, chunk], F32, tag=f"{tag}st2", name=f"{tag}st2", bufs=4
                )
                last_p2_dma = nc.sync.dma_start(
                    wst2[:], wt_d[k * 128 : (k + 1) * 128, j * chunk : (j + 1) * chunk]
                )
                if ci >= pass2_free_chunks:
                    # later chunks must not steal DMA bandwidth from the
                    # pass-1 max stream (which gates everything)
                    add_dep_helper(last_p2_dma.ins, macc_last.ins,
                                   reason="pass2 bulk re-DMA after pass1 max")
                nc.vector.tensor_scalar(
                    wst2[:], wst2[:], inv_s[:], C_RNE, op0=ALU.mult, op1=ALU.add
                )
                nc.vector.tensor_scalar(
                    dst_tiles[k][:, j * chunk : (j + 1) * chunk],
                    wst2[:], C_RNE, None, op0=ALU.subtract,
                )
            return scale, inv_s, macc_last

        w1q = [wq.tile([128, H], BF16, tag=f"w1q{d}", name=f"w1q{d}") for d in range(KD)]
        w2q = [wq.tile([128, D], BF16, tag=f"w2q{t}", name=f"w2q{t}") for t in range(KH)]

        def load_x_block(m0, m_blk):
            """DMA one x block (SWDGE — keeps it off the bulk HWDGE stream)
            and split into bf16 hi (+ lo)."""
            xh, xl = [], []
            for d in range(KD):
                xs_ = xstage.tile([128, m_blk], F32, tag=f"xs{d}", name=f"xs{d}")
                nc.gpsimd.dma_start(xs_[:], xt_d[d * 128 : (d + 1) * 128, m0 : m0 + m_blk])
                xh_ = xsplit.tile([128, m_blk], BF16, tag=f"xh{d}", name=f"xh{d}")
                nc.vector.tensor_copy(xh_[:], xs_[:])
                xh.append(xh_)
                if use_split:
                    xl_ = xsplit.tile([128, m_blk], BF16, tag=f"xl{d}", name=f"xl{d}")
                    nc.vector.tensor_tensor(xl_[:], xs_[:], xh_[:], op=ALU.subtract)
                    xl.append(xl_)
            return xh, xl

        s1, inv_s1, q1_macc = quantize(w1t_d, KD, H, 768, w1q, "q1",
                                       pass2_j_major=True, pass2_free_chunks=6)
        x0 = load_x_block(0, M_BLOCKS[0])
        # w2's bulk pass-1 stream must not steal DMA bandwidth from w1's
        # (which gates everything).
        s2, _, _ = quantize(w2t_d, KH, D, D, w2q, "q2",
                            gate_pass1_on=q1_macc.ins, pass2_free_chunks=KH)

        # b1' = b1 / s1   (per-partition column layout [128, KH])
        b1s = const.tile([128, KH], F32, tag="b1s")
        nc.vector.tensor_scalar(b1s[:], b1_pack[:], inv_s1[:], None, op0=ALU.mult)
        # c = s1 * s2  (final output scale), already per-partition [128,1]
        cscale = scal.tile([128, 1], F32, tag="cscale")
        nc.vector.tensor_tensor(cscale[:], s1[:], s2[:], op=ALU.mult)

        # ---------- main pipeline over row blocks ----------
        def fc1_block(m_blk, xh, xl):
            """fc1: hT[t] = relu(contract_d(w1q, xT) + b1')  (hidden on
            partitions).  Returns (hh, hl) bf16 hi/lo tiles."""
            hh, hl = [], []
            for t in range(KH):
                ps = ps1.tile([128, m_blk], F32, tag="ps1", name="ps1")
                total = KD * (2 if use_split else 1)
                cnt = 0
                for d in range(KD):
                    lhs = w1q[d][:, t * 128 : (t + 1) * 128]
                    nc.tensor.matmul(
                        ps[:], lhs, xh[d][:], start=(cnt == 0), stop=(cnt == total - 1)
                    )
                    cnt += 1
                    if use_split:
                        nc.tensor.matmul(
                            ps[:], lhs, xl[d][:], start=False, stop=(cnt == total - 1)
                        )
                        cnt += 1
                hf = hf32p.tile([128, m_blk], F32, tag="hf", name="hf")
                nc.scalar.activation(hf[:], ps[:], ACTF.Relu, bias=b1s[:, t : t + 1])
                hh_ = hpool.tile([128, m_blk], BF16, tag=f"hh{t}", name=f"hh{t}")
                nc.scalar.activation(hh_[:], hf[:], ACTF.Copy)
                hh.append(hh_)
                if use_split:
                    hl_ = hpool.tile([128, m_blk], BF16, tag=f"hl{t}", name=f"hl{t}")
                    nc.vector.tensor_tensor(hl_[:], hf[:], hh_[:], op=ALU.subtract)
                    hl.append(hl_)
            return hh, hl

        def fc2_block(m0, m_blk, hh, hl):
            """fc2 (transposed): outT[d, m] = c * contract_h(w2q, hT) + b2.
            w2q is the stationary operand, hT the moving one, so the row
            count only enters as streaming cycles (no N-penalty for the
            32-row tail) and the epilogue fuses scale+bias in one ACT op."""
            parts = [hh, hl] if use_split else [hh]
            for dt in range(KD):
                ps_ = ps2.tile([128, m_blk], F32, tag="ps2", name="ps2")
                total = KH * len(parts)
                cnt = 0
                for t in range(KH):
                    lhs = w2q[t][:, dt * 128 : (dt + 1) * 128]
                    for part in parts:
                        nc.tensor.matmul(
                            ps_[:], lhs, part[t][:],
                            start=(cnt == 0), stop=(cnt == total - 1),
                        )
                        cnt += 1
                ot = opool.tile([128, m_blk], F32, tag="ot", name="ot")
                # out = Identity(psum * c + b2)  — one ACT op
                nc.scalar.activation(
                    ot[:], ps_[:], ACTF.Identity,
                    bias=b2_pack[:, dt : dt + 1], scale=cscale[:],
                )
                nc.sync.dma_start(
                    out_d[dt * 128 : (dt + 1) * 128, m0 : m0 + m_blk], ot[:]
                )

        # Interleave: fc1 of block k+1 is emitted before fc2 of block k so
        # the PE always has fc1 work while fc2's inputs (w2q early on, h
        # tiles later) are still being produced.  h tiles are double
        # buffered (hpool bufs=2) to allow this.
        starts = []
        o = 0
        for mb in M_BLOCKS:
            starts.append(o)
            o += mb
        prev = None
        for blk, m_blk in enumerate(M_BLOCKS):
            xh, xl = x0 if blk == 0 else load_x_block(starts[blk], m_blk)
            hh, hl = fc1_block(m_blk, xh, xl)
            if prev is not None:
                fc2_block(starts[blk - 1], M_BLOCKS[blk - 1], *prev)
            prev = (hh, hl)
        fc2_block(starts[-1], M_BLOCKS[-1], *prev)

    if walrus_fixups:
        _split_oversized_waits(nc)
    return nc


_PROGRAM_CACHE = {}


def _get_program(qmax: float, use_split: bool = USE_SPLIT):
    key = (qmax, use_split)
    if key not in _PROGRAM_CACHE:
        _PROGRAM_CACHE[key] = build_program(qmax, use_split)
    return _PROGRAM_CACHE[key]


def kernel(x, w1, b1, w2, b2, bits):
    qmax = float(2.0 ** (int(bits) - 1) - 1.0)
    nc = _get_program(qmax)

    x = np.ascontiguousarray(np.asarray(x, dtype=np.float32)).reshape(M_TOTAL, D)
    w1t = np.ascontiguousarray(np.asarray(w1, dtype=np.float32).T)   # [768, 3072]
    w2t = np.ascontiguousarray(np.asarray(w2, dtype=np.float32).T)   # [3072, 768]
    b1h = np.ascontiguousarray(
        np.asarray(b1, dtype=np.float32).reshape(KH, 128).T
    )  # [128, KH]
    b2h = np.ascontiguousarray(
        np.asarray(b2, dtype=np.float32).reshape(KD, 128).T
    )  # [128, KD]
    xt_full = np.ascontiguousarray(x.T)                              # [768, 12544]

    ident = np.eye(128, dtype=np.float32)
    in_maps = []
    for c in range(N_CORES):
        xt_c = np.ascontiguousarray(xt_full[:, c * M_SHARD : (c + 1) * M_SHARD])
        in_maps.append(
            {"xt": xt_c, "w1t": w1t, "w2t": w2t, "b1": b1h, "b2": b2h,
             "ident": ident}
        )

    res = bass_utils.run_bass_kernel_spmd(nc, in_maps, core_ids=list(range(N_CORES)))
    out = np.concatenate(
        [res.results[c]["outT"].T for c in range(N_CORES)], axis=0
    )
    return np.ascontiguousarray(out.reshape(B, S, D))
